# revision 25
# baseline (speedup 1.0000x reference)
"""Trainium2 Bass kernel for the spectral-gating network (nn_DAPSO).

Model (B=4, C=256, H=W=256):
  - channels 0:128   : y_h = irfft(Gh * rfft(x, axis=H))   (per-channel gate)
  - channels 128:256 : y_w = irfft(Gw * rfft(x, axis=W))
  - gates Gh/Gw from tiny MLPs (computed on device)
  - channel attention: s = sigmoid(dw(gelu(W1 @ mean_hw(y) + b)))  -> y *= s
  - y2 = gelu(BN(lc_w @ y));  out = x + y2

Key algorithmic mapping: irfft(G*rfft(x)) along an axis of length N equals
T^T diag(ghat) T x with T the orthonormal real DFT basis (cos/sin rows), so
both branches become dense TensorE matmuls (no FFT).

Sharding: 8 cores = 4 batches x 2 w-halves. Each core computes BOTH branch
outputs for its (batch, w-half) spatial region.

v2 design: no DRAM scratch at all.  The HC branch keeps its *gated spectrum*
ubuf_h[k, w, c] resident in SBUF and the inverse transform is applied
just-in-time inside the conv loop, producing channel-major y columns.  The
WC branch keeps its *output* ybuf_w[c, h, w] resident (inverse applied per
h-row, channel-major).  The 1x1 conv then reads both directly from SBUF,
fuses BN+GELU and the residual add, and streams bf16 results out.

Channel-attention pooling uses the DC trick mean(irfft(G*rfft x)) =
G[0]*mean(x) for BOTH branches. The WC pool is computable locally (full w
on every core), so only one 512-byte AllReduce (HC partial sums) remains,
issued right after the HC forward pass and hidden behind the WC branch.

Per-core inputs (host-prepped, bf16):
  xh   (256,128,128)  [h, w, c]    HC-branch input slice (w-half)
  xw   (256,256,128)  [w, h, c]    WC-branch input (full w)
  xres (256,128,256)  [c, w, h]    residual slice
  out  (256,128,256)  [c, w, h]    bf16, host transposes/upcasts
"""
import sys
import os

sys.path.insert(0, "/opt/trn_rl_repo")

import numpy as np
import ml_dtypes

import concourse.bacc as bacc
import concourse.mybir as mybir
import concourse.tile as tile
from concourse import bass_utils

F32 = mybir.dt.float32
BF16 = mybir.dt.bfloat16
AF = mybir.ActivationFunctionType
ALU = mybir.AluOpType

N = 256          # H = W
C2 = 128         # channels per branch
B = 4
NCORES = 8
WS = 128         # per-core w-slice width

_BF16_NP = ml_dtypes.bfloat16

_MLPS = ("ah", "bc1", "aw", "bc2")


def _dft_basis():
    """Orthonormal real DFT basis T (N, N): y = T^T diag(ghat) T x == irfft(G*rfft(x))."""
    n = np.arange(N)
    k = np.arange(1, N // 2)
    T = np.zeros((N, N), np.float64)
    T[0, :] = 1.0 / np.sqrt(N)
    T[1:N // 2, :] = np.sqrt(2.0 / N) * np.cos(2 * np.pi * k[:, None] * n[None, :] / N)
    T[N // 2, :] = (1.0 / np.sqrt(N)) * ((-1.0) ** n)
    T[N // 2 + 1:, :] = np.sqrt(2.0 / N) * np.sin(2 * np.pi * k[:, None] * n[None, :] / N)
    return T.astype(np.float32)


def _part_major(a):
    """(256, ...) -> (128, 2, ...) partition-major layout."""
    a = np.asarray(a)
    return np.ascontiguousarray(a.reshape(2, 128, *a.shape[1:]).transpose(
        (1, 0) + tuple(range(2, a.ndim + 1))))


def _build():
    nc = bacc.Bacc("TRN2", target_bir_lowering=False, num_devices=NCORES)

    # ---------------- I/O declarations ----------------
    xh_d = nc.dram_tensor("xh", [256, 128, 128], BF16, kind="ExternalInput")
    xw_d = nc.dram_tensor("xw", [256, 256, 128], BF16, kind="ExternalInput")
    xres_d = nc.dram_tensor("xres", [256, 128, 256], BF16, kind="ExternalInput")
    tfwd_d = nc.dram_tensor("tfwd", [128, 2, 256], BF16, kind="ExternalInput")
    tinv_d = nc.dram_tensor("tinv", [128, 2, 256], BF16, kind="ExternalInput")
    tinvw_d = nc.dram_tensor("tinvw", [128, 2, 128], BF16, kind="ExternalInput")
    omega_d = nc.dram_tensor("omega", [1, 129], F32, kind="ExternalInput")
    lam_d = nc.dram_tensor("lam", [1, 128], F32, kind="ExternalInput")
    mlp_d = {}
    for m in ("o", "l"):  # stacked head pairs: o=(ah,aw) on omega, l=(bc1,bc2) on lam
        mlp_d[m] = dict(
            w1t=nc.dram_tensor(f"{m}_w1t", [1, 128], F32, kind="ExternalInput"),
            b1=nc.dram_tensor(f"{m}_b1v", [128, 1], F32, kind="ExternalInput"),
            w2t=nc.dram_tensor(f"{m}_w2t", [128, 128], F32, kind="ExternalInput"),
            b2=nc.dram_tensor(f"{m}_b2v", [128, 1], F32, kind="ExternalInput"),
            w3t=nc.dram_tensor(f"{m}_w3t", [128, 40], F32, kind="ExternalInput"),
            b3=nc.dram_tensor(f"{m}_b3v", [40, 1], F32, kind="ExternalInput"),
        )
    caw1t_d = nc.dram_tensor("caw1t", [128, 2, 256], F32, kind="ExternalInput")
    cab1_d = nc.dram_tensor("cab1", [128, 2], F32, kind="ExternalInput")
    dwc_d = nc.dram_tensor("dwc", [128, 2], F32, kind="ExternalInput")
    dwb_d = nc.dram_tensor("dwb", [128, 2], F32, kind="ExternalInput")
    lcwt_d = nc.dram_tensor("lcwt", [128, 2, 256], F32, kind="ExternalInput")
    bng_d = nc.dram_tensor("bng", [128, 2], F32, kind="ExternalInput")
    bnb_d = nc.dram_tensor("bnb", [128, 2], F32, kind="ExternalInput")
    bnm_d = nc.dram_tensor("bnm", [128, 2], F32, kind="ExternalInput")
    bnv_d = nc.dram_tensor("bnv", [128, 2], F32, kind="ExternalInput")

    out_d = nc.dram_tensor("out", [256, 128, 256], BF16, kind="ExternalOutput")

    arh_in = nc.dram_tensor("arh_in", [128, 1], F32)
    arh_out = nc.dram_tensor("arh_out", [128, 1], F32)
    bar_in = nc.dram_tensor("bar_in", [1, 1], F32)
    bar_out = nc.dram_tensor("bar_out", [1, 1], F32)

    with tile.TileContext(nc) as tc:
        with tc.tile_pool(name="consts", bufs=1) as consts, \
             tc.tile_pool(name="xin", bufs=2) as xin, \
             tc.tile_pool(name="uch", bufs=2) as uch, \
             tc.tile_pool(name="stg", bufs=2) as stg, \
             tc.tile_pool(name="crhs", bufs=2) as crhs, \
             tc.tile_pool(name="outp", bufs=2) as outp, \
             tc.tile_pool(name="gsb", bufs=1) as gsb, \
             tc.tile_pool(name="ps", bufs=1, space="PSUM") as ps:

            # Early sync barrier: absorbs cross-core launch skew on the
            # CC-cores while const loads / gate MLPs proceed, so the real
            # AllReduce later completes without waiting for a late peer.
            nc.gpsimd.collective_compute(
                "AllReduce", ALU.add,
                replica_groups=[[0, 1], [2, 3], [4, 5], [6, 7]],
                ins=[bar_in[:]], outs=[bar_out[:]])

            # ---------------- const loads (gate-MLP weights first) ----------
            mlp_t = {}
            for m in ("o", "l"):
                d = mlp_d[m]
                w1t = gsb.tile([1, 128], F32, tag=f"m_w1{m}")
                nc.sync.dma_start(out=w1t, in_=d["w1t"][:])
                b1 = gsb.tile([128, 1], F32, tag=f"m_b1{m}")
                nc.sync.dma_start(out=b1, in_=d["b1"][:])
                w2t = gsb.tile([128, 128], F32, tag=f"m_w2{m}")
                nc.sync.dma_start(out=w2t, in_=d["w2t"][:])
                b2 = gsb.tile([128, 1], F32, tag=f"m_b2{m}")
                nc.sync.dma_start(out=b2, in_=d["b2"][:])
                w3t = gsb.tile([128, 40], F32, tag=f"m_w3{m}")
                nc.sync.dma_start(out=w3t, in_=d["w3t"][:])
                b3 = gsb.tile([40, 1], F32, tag=f"m_b3{m}")
                nc.sync.dma_start(out=b3, in_=d["b3"][:])
                mlp_t[m] = (w1t, b1, w2t, b2, w3t, b3)
            omega_t = consts.tile([1, 129], F32, tag="omega")
            nc.sync.dma_start(out=omega_t, in_=omega_d[:])
            lam_t = consts.tile([1, 128], F32, tag="lam")
            nc.sync.dma_start(out=lam_t, in_=lam_d[:])
            tfwd_t = consts.tile([128, 2, 256], BF16, tag="tfwd")
            nc.sync.dma_start(out=tfwd_t, in_=tfwd_d[:])
            tinv_t = consts.tile([128, 2, 256], BF16, tag="tinv")
            nc.sync.dma_start(out=tinv_t, in_=tinv_d[:])
            tinvw_t = consts.tile([128, 2, 128], BF16, tag="tinvw")
            nc.sync.dma_start(out=tinvw_t, in_=tinvw_d[:])
            caw1t_t = consts.tile([128, 2, 256], F32, tag="caw1t")
            nc.sync.dma_start(out=caw1t_t, in_=caw1t_d[:])
            lcwt_t = consts.tile([128, 2, 256], F32, tag="lcwt")
            nc.sync.dma_start(out=lcwt_t, in_=lcwt_d[:])
            vec_t = {}
            for nm, d in (("cab1", cab1_d), ("dwc", dwc_d), ("dwb", dwb_d),
                          ("bng", bng_d), ("bnb", bnb_d), ("bnm", bnm_d), ("bnv", bnv_d)):
                vt = consts.tile([128, 2], F32, tag=f"v_{nm}")
                nc.sync.dma_start(out=vt, in_=d[:])
                vec_t[nm] = vt
            ones_t = consts.tile([128, 1], F32, tag="ones")
            nc.vector.memset(ones_t, 1.0)
            one1_t = consts.tile([1, 1], F32, tag="one1")
            nc.vector.memset(one1_t, 1.0)

            # resident state
            ubuf_h = consts.tile([128, 2, 128, 128], BF16, tag="ubufh")  # [k, kt, w, c]
            ybuf_w = consts.tile([128, 256, 128], BF16, tag="ybufw")     # [c, h, w]
            xsum_h = consts.tile([128, 2, 128], F32, tag="xsumh")        # [h, ht, c]
            xsum_w = consts.tile([128, 2, 128], F32, tag="xsumw")        # [w, wt, c]
            nc.vector.memset(xsum_h, 0.0)
            nc.vector.memset(xsum_w, 0.0)

            # ---------------- gate MLPs: two stacked (block-diag) heads ------
            def mlp_stacked(m, xvec, nk):
                w1t, b1, w2t, b2, w3t, b3 = mlp_t[m]
                p1 = ps.tile([128, nk], F32, tag="B0")
                nc.tensor.matmul(p1, lhsT=w1t, rhs=xvec, start=True, stop=True)
                h1 = gsb.tile([128, nk], F32, tag=f"m_h1{m}")
                nc.scalar.activation(h1, p1, AF.Gelu, bias=b1)
                p2 = ps.tile([128, nk], F32, tag="B1")
                nc.tensor.matmul(p2, lhsT=w2t, rhs=h1, start=True, stop=True)
                h2 = gsb.tile([128, nk], F32, tag=f"m_h2{m}")
                nc.scalar.activation(h2, p2, AF.Gelu, bias=b2)
                p3 = ps.tile([40, nk], F32, tag="B0")
                nc.tensor.matmul(p3, lhsT=w3t, rhs=h2, start=True, stop=True)
                at = gsb.tile([40, nk], F32, tag=f"m_at{m}")
                nc.scalar.activation(at, p3, AF.Identity, bias=b3)
                return at

            ghh = consts.tile([128, 2, 128], F32, tag="ghh")
            ghw = consts.tile([128, 2, 128], F32, tag="ghw")
            at_o = mlp_stacked("o", omega_t, 129)
            at_l = mlp_stacked("l", lam_t, 128)
            gp = {}
            gtags = {("h", 0): "A0", ("h", 1): "A1", ("w", 0): "B0", ("w", 1): "B1"}
            for (po, nmk) in ((0, "h"), (32, "w")):
                at = at_o[po:po + 8, :]
                bt = at_l[po:po + 8, :]
                g0 = ps.tile([128, 128], F32, tag=gtags[(nmk, 0)])
                nc.tensor.matmul(g0, lhsT=at[:, 0:128], rhs=bt, start=True, stop=True)
                gn = ps.tile([1, 128], F32, tag=gtags[(nmk, 1)])
                nc.tensor.matmul(gn, lhsT=at[:, 128:129], rhs=bt, start=True, stop=True)
                gp[(nmk, 0)] = g0
                gp[(nmk, 1)] = gn
            # softplus(z) = relu(z) + log1p(exp(-|z|)), stage-batched across all 4
            keys = list(gp.keys())
            sp = {}
            for i, key in enumerate(keys):
                npart = 128 if key[1] == 0 else 1
                na = gsb.tile([128, 128], F32, tag=f"sp_na{i}")
                nc.scalar.activation(na[:npart, :], gp[key], AF.Abs)
                sp[key] = na
            for i, key in enumerate(keys):
                npart = 128 if key[1] == 0 else 1
                ex = gsb.tile([128, 128], F32, tag=f"sp_ex{i}")
                nc.scalar.activation(ex[:npart, :], sp[key][:npart, :], AF.Exp, scale=-1.0)
                nc.vector.tensor_scalar_add(ex[:npart, :], ex[:npart, :], 1.0)
                sp[key] = ex
            for key in keys:
                npart = 128 if key[1] == 0 else 1
                nc.scalar.activation(sp[key][:npart, :], sp[key][:npart, :], AF.Ln)
            for i, key in enumerate(keys):
                npart = 128 if key[1] == 0 else 1
                re = gsb.tile([128, 128], F32, tag=f"sp_na{i}")
                nc.scalar.activation(re[:npart, :], gp[key], AF.Relu)
                gh = ghh if key[0] == "h" else ghw
                if key[1] == 0:
                    nc.vector.tensor_add(gh[:, 0, :], sp[key][:128, :], re[:128, :])
                else:
                    # rows 128+j of ghat equal G[j]: copy the aligned block first,
                    # then overwrite row 0 with the Nyquist G[128].
                    nc.vector.tensor_copy(gh[:, 1, :], gh[:, 0, :])
                    nc.vector.tensor_add(gh[0:1, 1, :], sp[key][0:1, :], re[0:1, :])
            for gh in (ghh, ghw):
                nc.vector.tensor_scalar_mul(gh[:, :, :], gh[:, :, :], float(8.0 ** -0.5))

            # ---------------- BN prep ----------------
            bninv = consts.tile([128, 2], F32, tag="bninv")
            nc.vector.tensor_scalar_add(bninv, vec_t["bnv"], 1e-5)
            nc.scalar.activation(bninv, bninv, AF.Sqrt)
            nc.vector.reciprocal(bninv, bninv)
            nc.vector.tensor_tensor(out=bninv, in0=vec_t["bng"], in1=bninv, op=ALU.mult)
            bnbeff = consts.tile([128, 2], F32, tag="bnbeff")
            nc.vector.tensor_tensor(out=bnbeff, in0=vec_t["bnm"], in1=bninv, op=ALU.mult)
            nc.vector.tensor_tensor(out=bnbeff, in0=vec_t["bnb"], in1=bnbeff, op=ALU.subtract)

            # ---------------- HC forward: gated spectrum -> ubuf_h ----------------
            for wb in range(16):
                w0 = wb * 8
                xt = []
                for ht in (0, 1):
                    t = xin.tile([128, 8, 128], BF16, tag=f"xa{ht}")
                    nc.sync.dma_start(out=t, in_=xh_d[ht * 128:(ht + 1) * 128, w0:w0 + 8, :])
                    xt.append(t)
                    # accumulate xsum_h over w via tree-adds (contiguous)
                    t1 = gsb.tile([128, 4, 128], BF16, tag="th10")
                    nc.vector.tensor_tensor(out=t1, in0=t[:, 0:4, :], in1=t[:, 4:8, :], op=ALU.add)
                    t2 = gsb.tile([128, 2, 128], BF16, tag="th20")
                    nc.vector.tensor_tensor(out=t2, in0=t1[:, 0:2, :], in1=t1[:, 2:4, :], op=ALU.add)
                    t3 = gsb.tile([128, 1, 128], BF16, tag="th30")
                    nc.vector.tensor_tensor(out=t3, in0=t2[:, 0:1, :], in1=t2[:, 1:2, :], op=ALU.add)
                    nc.vector.tensor_tensor(out=xsum_h[:, ht, :], in0=xsum_h[:, ht, :],
                                            in1=t3[:, 0, :], op=ALU.add)
                pks = []
                for kt in (0, 1):
                    pk = ps.tile([128, 8, 128], F32, tag=f"{'AB'[wb % 2]}{kt}")
                    for bh in (0, 1):  # one PSUM bank (2KB) per matmul group
                        for ht in (0, 1):
                            nc.tensor.matmul(pk[:, bh * 4:bh * 4 + 4, :],
                                             lhsT=tfwd_t[:, ht, kt * 128:(kt + 1) * 128],
                                             rhs=xt[ht][:, bh * 4:bh * 4 + 4, :],
                                             start=(ht == 0), stop=(ht == 1))
                    pks.append(pk)
                for kt in (0, 1):
                    nc.vector.tensor_tensor(
                        out=ubuf_h[:, kt, w0:w0 + 8, :], in0=pks[kt],
                        in1=ghh[:, kt, :].unsqueeze(1).broadcast_to([128, 8, 128]),
                        op=ALU.mult)

            # ---------------- WC branch: spectrum + inverse -> ybuf_w ----------------
            for hb in range(32):
                h0 = hb * 8
                if hb == 1:
                    # HC pool partial = sum_{h, w-half} x -> single tiny
                    # AllReduce, issued here so PE flows into WC block 0 first
                    ph_ps = ps.tile([128, 1], F32, tag="B0")
                    for ht in (0, 1):
                        nc.tensor.matmul(ph_ps, lhsT=xsum_h[:, ht, :], rhs=ones_t,
                                         start=(ht == 0), stop=(ht == 1))
                    poolh_sb = gsb.tile([128, 1], F32, tag="poolh")
                    nc.vector.tensor_copy(poolh_sb, ph_ps)
                    nc.scalar.dma_start(out=arh_in[:], in_=poolh_sb)
                    nc.gpsimd.collective_compute(
                        "AllReduce", ALU.add,
                        replica_groups=[[0, 1], [2, 3], [4, 5], [6, 7]],
                        ins=[arh_in[:]], outs=[arh_out[:]])
                xt = []
                for wt in (0, 1):
                    t = xin.tile([128, 8, 128], BF16, tag=f"xb{wt}")
                    nc.sync.dma_start(out=t, in_=xw_d[wt * 128:(wt + 1) * 128, h0:h0 + 8, :])
                    xt.append(t)
                    t1 = gsb.tile([128, 4, 128], BF16, tag="th10")
                    nc.vector.tensor_tensor(out=t1, in0=t[:, 0:4, :], in1=t[:, 4:8, :], op=ALU.add)
                    t2 = gsb.tile([128, 2, 128], BF16, tag="th20")
                    nc.vector.tensor_tensor(out=t2, in0=t1[:, 0:2, :], in1=t1[:, 2:4, :], op=ALU.add)
                    t3 = gsb.tile([128, 1, 128], BF16, tag="th30")
                    nc.vector.tensor_tensor(out=t3, in0=t2[:, 0:1, :], in1=t2[:, 1:2, :], op=ALU.add)
                    nc.vector.tensor_tensor(out=xsum_w[:, wt, :], in0=xsum_w[:, wt, :],
                                            in1=t3[:, 0, :], op=ALU.add)
                uts = []
                for kt in (0, 1):
                    pk = ps.tile([128, 8, 128], F32, tag=f"A{kt}")
                    for bh in (0, 1):
                        for wt in (0, 1):
                            nc.tensor.matmul(pk[:, bh * 4:bh * 4 + 4, :],
                                             lhsT=tfwd_t[:, wt, kt * 128:(kt + 1) * 128],
                                             rhs=xt[wt][:, bh * 4:bh * 4 + 4, :],
                                             start=(wt == 0), stop=(wt == 1))
                    ut = uch.tile([128, 8, 128], BF16, tag=f"u{kt}")
                    nc.vector.tensor_tensor(
                        out=ut, in0=pk,
                        in1=ghw[:, kt, :].unsqueeze(1).broadcast_to([128, 8, 128]),
                        op=ALU.mult)
                    uts.append(ut)
                for half in (0, 1):
                    pw = ps.tile([128, 4, 128], F32, tag=f"B{half}")
                    for i in range(4):
                        hh = half * 4 + i
                        for kt in (0, 1):
                            nc.tensor.matmul(pw[:, i, :], lhsT=uts[kt][:, hh, :],
                                             rhs=tinvw_t[:, kt, :],
                                             start=(kt == 0), stop=(kt == 1))
                    if half == 0:
                        nc.scalar.activation(
                            ybuf_w[:, h0 + half * 4:h0 + half * 4 + 4, :], pw, AF.Copy)
                    else:
                        nc.vector.tensor_copy(
                            ybuf_w[:, h0 + half * 4:h0 + half * 4 + 4, :], pw)

            # ---------------- pools + channel attention -> folded conv weights ----
            pw_ps = ps.tile([128, 1], F32, tag="A0")
            for wt in (0, 1):
                nc.tensor.matmul(pw_ps, lhsT=xsum_w[:, wt, :], rhs=ones_t,
                                 start=(wt == 0), stop=(wt == 1))
            g0h_ps = ps.tile([128, 1], F32, tag="A1")
            nc.tensor.matmul(g0h_ps, lhsT=ghh[0:1, 0, :], rhs=one1_t, start=True, stop=True)
            g0w_ps = ps.tile([128, 1], F32, tag="B0")
            nc.tensor.matmul(g0w_ps, lhsT=ghw[0:1, 0, :], rhs=one1_t, start=True, stop=True)

            arh_sb = gsb.tile([128, 1], F32, tag="arhsb")
            nc.sync.dma_start(out=arh_sb, in_=arh_out[:])
            p_sb = []
            p0 = gsb.tile([128, 1], F32, tag="p0")
            nc.vector.tensor_tensor(out=p0, in0=arh_sb, in1=g0h_ps, op=ALU.mult)
            p_sb.append(p0)
            g0w_sb = gsb.tile([128, 1], F32, tag="g0wsb")
            nc.vector.tensor_copy(g0w_sb, g0w_ps)
            p1 = gsb.tile([128, 1], F32, tag="p1")
            nc.vector.tensor_tensor(out=p1, in0=pw_ps, in1=g0w_sb, op=ALU.mult)
            p_sb.append(p1)

            q_sb = []
            for ot in (0, 1):
                q_ps = ps.tile([128, 1], F32, tag=f"B{ot}")
                for ct in (0, 1):
                    nc.tensor.matmul(q_ps, lhsT=caw1t_t[:, ct, ot * 128:(ot + 1) * 128],
                                     rhs=p_sb[ct], start=(ct == 0), stop=(ct == 1))
                qt = gsb.tile([128, 1], F32, tag=f"q{ot}")
                nc.scalar.activation(qt, q_ps, AF.Gelu, bias=vec_t["cab1"][:, ot:ot + 1])
                nc.vector.tensor_tensor(out=qt, in0=qt, in1=vec_t["dwc"][:, ot:ot + 1],
                                        op=ALU.mult)
                q_sb.append(qt)
            s_sb = []
            for ot in (0, 1):
                s_t = gsb.tile([128, 1], F32, tag=f"s{ot}")
                nc.scalar.activation(s_t, q_sb[ot], AF.Sigmoid, bias=vec_t["dwb"][:, ot:ot + 1])
                s_sb.append(s_t)
            wsc = consts.tile([128, 2, 256], BF16, tag="wsc")
            for ct in (0, 1):
                nc.vector.tensor_scalar_mul(wsc[:, ct, :], lcwt_t[:, ct, :], s_sb[ct])

            # ---------------- conv 1x1 + BN + GELU + residual, streamed out ------
            for ch in range(32):
                w0 = ch * 4
                # JIT inverse of the HC spectrum: y_h columns, channel-major
                py = ps.tile([128, 4, 256], F32, tag=f"A{ch % 2}")
                for i in range(4):
                    for kt in (0, 1):
                        nc.tensor.matmul(py[:, i, :], lhsT=ubuf_h[:, kt, w0 + i, :],
                                         rhs=tinv_t[:, kt, :],
                                         start=(kt == 0), stop=(kt == 1))
                yt = stg.tile([128, 4, 256], BF16, tag="yt")
                nc.vector.tensor_copy(yt, py)
                xrs = []
                for ot in (0, 1):
                    xr = crhs.tile([128, 4, 256], BF16, tag=f"xr{ot}")
                    nc.sync.dma_start(out=xr,
                                      in_=xres_d[ot * 128:(ot + 1) * 128, w0:w0 + 4, :])
                    xrs.append(xr)
                for ot in (0, 1):
                    po = ps.tile([128, 4, 256], F32, tag=f"B{ot}")
                    for wh in (0, 1):
                        pos = po[:, wh * 2:wh * 2 + 2, :]
                        nc.tensor.matmul(pos, lhsT=wsc[:, 0, ot * 128:(ot + 1) * 128],
                                         rhs=yt[:, wh * 2:wh * 2 + 2, :],
                                         start=True, stop=False)
                        nc.tensor.matmul(pos, lhsT=wsc[:, 1, ot * 128:(ot + 1) * 128],
                                         rhs=ybuf_w[:, :, w0 + wh * 2:w0 + wh * 2 + 2]
                                         .transpose([0, 2, 1]),
                                         start=False, stop=True)
                    ga = outp.tile([128, 4, 256], BF16, tag=f"ga{ot}")
                    nc.scalar.activation(ga, po, AF.Gelu,
                                         bias=bnbeff[:, ot:ot + 1],
                                         scale=bninv[:, ot:ot + 1])
                    og = outp.tile([128, 4, 256], BF16, tag=f"og{ot}")
                    nc.vector.tensor_tensor(out=og, in0=ga, in1=xrs[ot], op=ALU.add)
                    eng = nc.scalar if ot == 0 else nc.gpsimd
                    eng.dma_start(out=out_d[ot * 128:(ot + 1) * 128, w0:w0 + 4, :],
                                  in_=og)

    nc.compile()
    return nc


_NC_CACHE = None


def _get_nc():
    global _NC_CACHE
    if _NC_CACHE is None:
        _NC_CACHE = _build()
    return _NC_CACHE


def _host_consts(inputs, core):
    """Per-core constant inputs (everything except the x shards)."""
    s = core % 2
    wlo = WS * s
    T = _dft_basis()
    d = {}
    d["tfwd"] = _part_major(np.ascontiguousarray(T.T)).astype(_BF16_NP)
    d["tinv"] = _part_major(T).astype(_BF16_NP)
    d["tinvw"] = _part_major(np.ascontiguousarray(T[:, wlo:wlo + WS])).astype(_BF16_NP)
    d["omega"] = (np.arange(129, dtype=np.float32) / 128.0 - 1.0).reshape(1, 129)
    d["lam"] = np.linspace(-1.0, 1.0, 128, dtype=np.float32).reshape(1, 128)
    for m, (ha, hb) in (("o", ("ah", "aw")), ("l", ("bc1", "bc2"))):
        w1 = np.zeros((1, 128), np.float32)
        w1[0, 0:64] = inputs[f"{ha}_w1"][:, 0]
        w1[0, 64:128] = inputs[f"{hb}_w1"][:, 0]
        d[f"{m}_w1t"] = w1
        b1 = np.zeros((128, 1), np.float32)
        b1[0:64, 0] = inputs[f"{ha}_b1"]
        b1[64:128, 0] = inputs[f"{hb}_b1"]
        d[f"{m}_b1v"] = b1
        w2 = np.zeros((128, 128), np.float32)
        w2[0:64, 0:64] = inputs[f"{ha}_w2"].T
        w2[64:128, 64:128] = inputs[f"{hb}_w2"].T
        d[f"{m}_w2t"] = w2
        b2 = np.zeros((128, 1), np.float32)
        b2[0:64, 0] = inputs[f"{ha}_b2"]
        b2[64:128, 0] = inputs[f"{hb}_b2"]
        d[f"{m}_b2v"] = b2
        w3 = np.zeros((128, 40), np.float32)
        w3[0:64, 0:8] = inputs[f"{ha}_w3"].T
        w3[64:128, 32:40] = inputs[f"{hb}_w3"].T
        d[f"{m}_w3t"] = w3
        b3 = np.zeros((40, 1), np.float32)
        b3[0:8, 0] = inputs[f"{ha}_b3"]
        b3[32:40, 0] = inputs[f"{hb}_b3"]
        d[f"{m}_b3v"] = b3
    d["caw1t"] = _part_major(np.ascontiguousarray(inputs["ca_w1"].T) / 65536.0).astype(np.float32)
    d["cab1"] = _part_major(inputs["ca_b1"]).astype(np.float32)
    d["dwc"] = _part_major(np.ascontiguousarray(inputs["ca_dw"][:, 1, 1])).astype(np.float32)
    d["dwb"] = _part_major(inputs["ca_db"]).astype(np.float32)
    d["lcwt"] = _part_major(np.ascontiguousarray(inputs["lc_w"].T)).astype(np.float32)
    d["bng"] = _part_major(inputs["bn_g"]).astype(np.float32)
    d["bnb"] = _part_major(inputs["bn_b"]).astype(np.float32)
    d["bnm"] = _part_major(inputs["bn_m"]).astype(np.float32)
    d["bnv"] = _part_major(inputs["bn_v"]).astype(np.float32)
    return d


def kernel(**inputs):
    x = np.asarray(inputs["x"], np.float32)
    nc = _get_nc()

    in_maps = []
    for core in range(NCORES):
        b, s = core // 2, core % 2
        wlo = WS * s
        m = _host_consts(inputs, core)
        # xh[h, w, c] = x[b, c, h, wlo+w]
        m["xh"] = np.ascontiguousarray(
            x[b, :C2, :, wlo:wlo + WS].transpose(1, 2, 0)).astype(_BF16_NP)
        # xw[w, h, c] = x[b, 128+c, h, w]
        m["xw"] = np.ascontiguousarray(
            x[b, C2:, :, :].transpose(2, 1, 0)).astype(_BF16_NP)
        # xres[c, w, h] = x[b, c, h, wlo+w]
        m["xres"] = np.ascontiguousarray(
            x[b, :, :, wlo:wlo + WS].transpose(0, 2, 1)).astype(_BF16_NP)
        in_maps.append(m)

    trace = os.environ.get("BASS_KERNEL_TRACE", "0") == "1"
    res = bass_utils.run_bass_kernel_spmd(
        nc, in_maps, core_ids=list(range(NCORES)),
        trace=trace, trace_cores=list(range(NCORES)) if trace else None,
        stitch_traces=False)
    if trace and res.exec_time_ns is not None:
        print(f"HW exec time: {res.exec_time_ns} ns")
        print(f"   mean exec time: {res.mean_exec_time_ns} ns  "
              f"(slowest core {res.max_exec_time_core_id})")
        if res.instructions_and_trace is not None:
            print("   trace:", res.instructions_and_trace[1])

    out = np.empty((B, 2 * C2, N, N), np.float32)
    for core in range(NCORES):
        b, s = core // 2, core % 2
        wlo = WS * s
        # device out is [c, w, h] bf16
        out[b, :, :, wlo:wlo + WS] = np.asarray(
            res.results[core]["out"], np.float32).transpose(0, 2, 1)
    return out


# revision 26
# speedup vs baseline: 1.0399x; 1.0399x over previous
"""Trainium2 Bass kernel for the spectral-gating network (nn_DAPSO).

Model (B=4, C=256, H=W=256):
  - channels 0:128   : y_h = irfft(Gh * rfft(x, axis=H))   (per-channel gate)
  - channels 128:256 : y_w = irfft(Gw * rfft(x, axis=W))
  - gates Gh/Gw from tiny MLPs (computed on device)
  - channel attention: s = sigmoid(dw(gelu(W1 @ mean_hw(y) + b)))  -> y *= s
  - y2 = gelu(BN(lc_w @ y));  out = x + y2

Key algorithmic mapping: irfft(G*rfft(x)) along an axis of length N equals
T^T diag(ghat) T x with T the orthonormal real DFT basis (cos/sin rows), so
both branches become dense TensorE matmuls (no FFT).

Sharding: 8 cores = 4 batches x 2 w-halves. Each core computes BOTH branch
outputs for its (batch, w-half) spatial region.

v2 design: no DRAM scratch at all.  The HC branch keeps its *gated spectrum*
ubuf_h[k, w, c] resident in SBUF and the inverse transform is applied
just-in-time inside the conv loop, producing channel-major y columns.  The
WC branch keeps its *output* ybuf_w[c, h, w] resident (inverse applied per
h-row, channel-major).  The 1x1 conv then reads both directly from SBUF,
fuses BN+GELU and the residual add, and streams bf16 results out.

Channel-attention pooling uses the DC trick mean(irfft(G*rfft x)) =
G[0]*mean(x) for BOTH branches. The WC pool is computable locally (full w
on every core), so only one 512-byte AllReduce (HC partial sums) remains,
issued right after the HC forward pass and hidden behind the WC branch.

Per-core inputs (host-prepped, bf16):
  xh   (256,128,128)  [h, w, c]    HC-branch input slice (w-half)
  xw   (256,256,128)  [w, h, c]    WC-branch input (full w)
  xres (256,128,256)  [c, w, h]    residual slice
  out  (256,128,256)  [c, w, h]    bf16, host transposes/upcasts
"""
import sys
import os

sys.path.insert(0, "/opt/trn_rl_repo")

import numpy as np
import ml_dtypes

import concourse.bacc as bacc
import concourse.mybir as mybir
import concourse.tile as tile
from concourse import bass_utils

F32 = mybir.dt.float32
BF16 = mybir.dt.bfloat16
AF = mybir.ActivationFunctionType
ALU = mybir.AluOpType

N = 256          # H = W
C2 = 128         # channels per branch
B = 4
NCORES = 8
WS = 128         # per-core w-slice width

_BF16_NP = ml_dtypes.bfloat16

_MLPS = ("ah", "bc1", "aw", "bc2")


def _dft_basis():
    """Orthonormal real DFT basis T (N, N): y = T^T diag(ghat) T x == irfft(G*rfft(x))."""
    n = np.arange(N)
    k = np.arange(1, N // 2)
    T = np.zeros((N, N), np.float64)
    T[0, :] = 1.0 / np.sqrt(N)
    T[1:N // 2, :] = np.sqrt(2.0 / N) * np.cos(2 * np.pi * k[:, None] * n[None, :] / N)
    T[N // 2, :] = (1.0 / np.sqrt(N)) * ((-1.0) ** n)
    T[N // 2 + 1:, :] = np.sqrt(2.0 / N) * np.sin(2 * np.pi * k[:, None] * n[None, :] / N)
    return T.astype(np.float32)


def _part_major(a):
    """(256, ...) -> (128, 2, ...) partition-major layout."""
    a = np.asarray(a)
    return np.ascontiguousarray(a.reshape(2, 128, *a.shape[1:]).transpose(
        (1, 0) + tuple(range(2, a.ndim + 1))))


def _build():
    nc = bacc.Bacc("TRN2", target_bir_lowering=False, num_devices=NCORES)

    # ---------------- I/O declarations ----------------
    xh_d = nc.dram_tensor("xh", [256, 128, 128], BF16, kind="ExternalInput")
    xw_d = nc.dram_tensor("xw", [256, 256, 128], BF16, kind="ExternalInput")
    xres_d = nc.dram_tensor("xres", [256, 128, 256], BF16, kind="ExternalInput")
    tfwd_d = nc.dram_tensor("tfwd", [128, 2, 256], BF16, kind="ExternalInput")
    tinv_d = nc.dram_tensor("tinv", [128, 2, 256], BF16, kind="ExternalInput")
    tinvw_d = nc.dram_tensor("tinvw", [128, 2, 128], BF16, kind="ExternalInput")
    omega_d = nc.dram_tensor("omega", [1, 129], F32, kind="ExternalInput")
    lam_d = nc.dram_tensor("lam", [1, 128], F32, kind="ExternalInput")
    mlp_d = {}
    for m in ("o", "l"):  # stacked head pairs: o=(ah,aw) on omega, l=(bc1,bc2) on lam
        mlp_d[m] = dict(
            w1t=nc.dram_tensor(f"{m}_w1t", [1, 128], F32, kind="ExternalInput"),
            b1=nc.dram_tensor(f"{m}_b1v", [128, 1], F32, kind="ExternalInput"),
            w2t=nc.dram_tensor(f"{m}_w2t", [128, 128], F32, kind="ExternalInput"),
            b2=nc.dram_tensor(f"{m}_b2v", [128, 1], F32, kind="ExternalInput"),
            w3t=nc.dram_tensor(f"{m}_w3t", [128, 40], F32, kind="ExternalInput"),
            b3=nc.dram_tensor(f"{m}_b3v", [40, 1], F32, kind="ExternalInput"),
        )
    caw1t_d = nc.dram_tensor("caw1t", [128, 2, 256], F32, kind="ExternalInput")
    cab1_d = nc.dram_tensor("cab1", [128, 2], F32, kind="ExternalInput")
    dwc_d = nc.dram_tensor("dwc", [128, 2], F32, kind="ExternalInput")
    dwb_d = nc.dram_tensor("dwb", [128, 2], F32, kind="ExternalInput")
    lcwt_d = nc.dram_tensor("lcwt", [128, 2, 256], F32, kind="ExternalInput")
    bng_d = nc.dram_tensor("bng", [128, 2], F32, kind="ExternalInput")
    bnb_d = nc.dram_tensor("bnb", [128, 2], F32, kind="ExternalInput")
    bnm_d = nc.dram_tensor("bnm", [128, 2], F32, kind="ExternalInput")
    bnv_d = nc.dram_tensor("bnv", [128, 2], F32, kind="ExternalInput")

    out_d = nc.dram_tensor("out", [256, 128, 256], BF16, kind="ExternalOutput")

    arh_in = nc.dram_tensor("arh_in", [128, 1], F32)
    arh_out = nc.dram_tensor("arh_out", [128, 1], F32)
    bar_in = nc.dram_tensor("bar_in", [1, 1], F32)
    bar_out = nc.dram_tensor("bar_out", [1, 1], F32)

    with tile.TileContext(nc) as tc:
        with tc.tile_pool(name="consts", bufs=1) as consts, \
             tc.tile_pool(name="xin", bufs=2) as xin, \
             tc.tile_pool(name="uch", bufs=2) as uch, \
             tc.tile_pool(name="stg", bufs=2) as stg, \
             tc.tile_pool(name="crhs", bufs=2) as crhs, \
             tc.tile_pool(name="outp", bufs=2) as outp, \
             tc.tile_pool(name="gsb", bufs=1) as gsb, \
             tc.tile_pool(name="ps", bufs=1, space="PSUM") as ps:

            # Early sync barrier: absorbs cross-core launch skew on the
            # CC-cores while const loads / gate MLPs proceed, so the real
            # AllReduce later completes without waiting for a late peer.
            nc.gpsimd.collective_compute(
                "AllReduce", ALU.add,
                replica_groups=[[0, 1], [2, 3], [4, 5], [6, 7]],
                ins=[bar_in[:]], outs=[bar_out[:]])

            # ---------------- const loads (gate-MLP weights first) ----------
            mlp_t = {}
            for m in ("o", "l"):
                d = mlp_d[m]
                w1t = gsb.tile([1, 128], F32, tag=f"m_w1{m}")
                nc.sync.dma_start(out=w1t, in_=d["w1t"][:])
                b1 = gsb.tile([128, 1], F32, tag=f"m_b1{m}")
                nc.sync.dma_start(out=b1, in_=d["b1"][:])
                w2t = gsb.tile([128, 128], F32, tag=f"m_w2{m}")
                nc.sync.dma_start(out=w2t, in_=d["w2t"][:])
                b2 = gsb.tile([128, 1], F32, tag=f"m_b2{m}")
                nc.sync.dma_start(out=b2, in_=d["b2"][:])
                w3t = gsb.tile([128, 40], F32, tag=f"m_w3{m}")
                nc.sync.dma_start(out=w3t, in_=d["w3t"][:])
                b3 = gsb.tile([40, 1], F32, tag=f"m_b3{m}")
                nc.sync.dma_start(out=b3, in_=d["b3"][:])
                mlp_t[m] = (w1t, b1, w2t, b2, w3t, b3)
            omega_t = consts.tile([1, 129], F32, tag="omega")
            nc.sync.dma_start(out=omega_t, in_=omega_d[:])
            lam_t = consts.tile([1, 128], F32, tag="lam")
            nc.sync.dma_start(out=lam_t, in_=lam_d[:])
            tfwd_t = consts.tile([128, 2, 256], BF16, tag="tfwd")
            nc.sync.dma_start(out=tfwd_t, in_=tfwd_d[:])
            tinv_t = consts.tile([128, 2, 256], BF16, tag="tinv")
            nc.sync.dma_start(out=tinv_t, in_=tinv_d[:])
            tinvw_t = consts.tile([128, 2, 128], BF16, tag="tinvw")
            nc.sync.dma_start(out=tinvw_t, in_=tinvw_d[:])
            caw1t_t = consts.tile([128, 2, 256], F32, tag="caw1t")
            nc.sync.dma_start(out=caw1t_t, in_=caw1t_d[:])
            lcwt_t = consts.tile([128, 2, 256], F32, tag="lcwt")
            nc.sync.dma_start(out=lcwt_t, in_=lcwt_d[:])
            vec_t = {}
            for nm, d in (("cab1", cab1_d), ("dwc", dwc_d), ("dwb", dwb_d),
                          ("bng", bng_d), ("bnb", bnb_d), ("bnm", bnm_d), ("bnv", bnv_d)):
                vt = consts.tile([128, 2], F32, tag=f"v_{nm}")
                nc.sync.dma_start(out=vt, in_=d[:])
                vec_t[nm] = vt
            ones_t = consts.tile([128, 1], F32, tag="ones")
            nc.vector.memset(ones_t, 1.0)
            one1_t = consts.tile([1, 1], F32, tag="one1")
            nc.vector.memset(one1_t, 1.0)

            # resident state
            ubuf_h = consts.tile([128, 2, 128, 128], BF16, tag="ubufh")  # [k, kt, w, c]
            ybuf_w = consts.tile([128, 256, 128], BF16, tag="ybufw")     # [c, h, w]
            xsum_h = consts.tile([128, 2, 128], F32, tag="xsumh")        # [h, ht, c]
            xsum_w = consts.tile([128, 2, 128], F32, tag="xsumw")        # [w, wt, c]
            nc.vector.memset(xsum_h, 0.0)
            nc.vector.memset(xsum_w, 0.0)

            # ---------------- gate MLPs: two stacked (block-diag) heads ------
            def mlp_stacked(m, xvec, nk):
                w1t, b1, w2t, b2, w3t, b3 = mlp_t[m]
                p1 = ps.tile([128, nk], F32, tag="B0")
                nc.tensor.matmul(p1, lhsT=w1t, rhs=xvec, start=True, stop=True)
                h1 = gsb.tile([128, nk], F32, tag=f"m_h1{m}")
                nc.scalar.activation(h1, p1, AF.Gelu, bias=b1)
                p2 = ps.tile([128, nk], F32, tag="B1")
                nc.tensor.matmul(p2, lhsT=w2t, rhs=h1, start=True, stop=True)
                h2 = gsb.tile([128, nk], F32, tag=f"m_h2{m}")
                nc.scalar.activation(h2, p2, AF.Gelu, bias=b2)
                p3 = ps.tile([40, nk], F32, tag="B0")
                nc.tensor.matmul(p3, lhsT=w3t, rhs=h2, start=True, stop=True)
                at = gsb.tile([40, nk], F32, tag=f"m_at{m}")
                nc.scalar.activation(at, p3, AF.Identity, bias=b3)
                return at

            ghh = consts.tile([128, 2, 128], F32, tag="ghh")
            ghw = consts.tile([128, 2, 128], F32, tag="ghw")
            at_o = mlp_stacked("o", omega_t, 129)
            at_l = mlp_stacked("l", lam_t, 128)
            gp = {}
            gtags = {("h", 0): "A0", ("h", 1): "A1", ("w", 0): "B0", ("w", 1): "B1"}
            for (po, nmk) in ((0, "h"), (32, "w")):
                at = at_o[po:po + 8, :]
                bt = at_l[po:po + 8, :]
                g0 = ps.tile([128, 128], F32, tag=gtags[(nmk, 0)])
                nc.tensor.matmul(g0, lhsT=at[:, 0:128], rhs=bt, start=True, stop=True)
                gn = ps.tile([1, 128], F32, tag=gtags[(nmk, 1)])
                nc.tensor.matmul(gn, lhsT=at[:, 128:129], rhs=bt, start=True, stop=True)
                gp[(nmk, 0)] = g0
                gp[(nmk, 1)] = gn
            # softplus(z) = relu(z) + log1p(exp(-|z|)), stage-batched across all 4
            keys = list(gp.keys())
            sp = {}
            for i, key in enumerate(keys):
                npart = 128 if key[1] == 0 else 1
                na = gsb.tile([128, 128], F32, tag=f"sp_na{i}")
                nc.scalar.activation(na[:npart, :], gp[key], AF.Abs)
                sp[key] = na
            for i, key in enumerate(keys):
                npart = 128 if key[1] == 0 else 1
                ex = gsb.tile([128, 128], F32, tag=f"sp_ex{i}")
                nc.scalar.activation(ex[:npart, :], sp[key][:npart, :], AF.Exp, scale=-1.0)
                nc.vector.tensor_scalar_add(ex[:npart, :], ex[:npart, :], 1.0)
                sp[key] = ex
            for key in keys:
                npart = 128 if key[1] == 0 else 1
                nc.scalar.activation(sp[key][:npart, :], sp[key][:npart, :], AF.Ln)
            for i, key in enumerate(keys):
                npart = 128 if key[1] == 0 else 1
                re = gsb.tile([128, 128], F32, tag=f"sp_na{i}")
                nc.scalar.activation(re[:npart, :], gp[key], AF.Relu)
                gh = ghh if key[0] == "h" else ghw
                if key[1] == 0:
                    nc.vector.tensor_add(gh[:, 0, :], sp[key][:128, :], re[:128, :])
                else:
                    # rows 128+j of ghat equal G[j]: copy the aligned block first,
                    # then overwrite row 0 with the Nyquist G[128].
                    nc.vector.tensor_copy(gh[:, 1, :], gh[:, 0, :])
                    nc.vector.tensor_add(gh[0:1, 1, :], sp[key][0:1, :], re[0:1, :])
            for gh in (ghh, ghw):
                nc.vector.tensor_scalar_mul(gh[:, :, :], gh[:, :, :], float(8.0 ** -0.5))

            # ---------------- BN prep ----------------
            bninv = consts.tile([128, 2], F32, tag="bninv")
            nc.vector.tensor_scalar_add(bninv, vec_t["bnv"], 1e-5)
            nc.scalar.activation(bninv, bninv, AF.Sqrt)
            nc.vector.reciprocal(bninv, bninv)
            nc.vector.tensor_tensor(out=bninv, in0=vec_t["bng"], in1=bninv, op=ALU.mult)
            bnbeff = consts.tile([128, 2], F32, tag="bnbeff")
            nc.vector.tensor_tensor(out=bnbeff, in0=vec_t["bnm"], in1=bninv, op=ALU.mult)
            nc.vector.tensor_tensor(out=bnbeff, in0=vec_t["bnb"], in1=bnbeff, op=ALU.subtract)

            # ---------------- HC forward: gated spectrum -> ubuf_h ----------------
            for wb in range(16):
                w0 = wb * 8
                xt = []
                for ht in (0, 1):
                    t = xin.tile([128, 8, 128], BF16, tag=f"xa{ht}")
                    nc.sync.dma_start(out=t, in_=xh_d[ht * 128:(ht + 1) * 128, w0:w0 + 8, :])
                    xt.append(t)
                    # accumulate xsum_h over w via tree-adds (contiguous)
                    t1 = gsb.tile([128, 4, 128], BF16, tag="th10")
                    nc.vector.tensor_tensor(out=t1, in0=t[:, 0:4, :], in1=t[:, 4:8, :], op=ALU.add)
                    t2 = gsb.tile([128, 2, 128], BF16, tag="th20")
                    nc.vector.tensor_tensor(out=t2, in0=t1[:, 0:2, :], in1=t1[:, 2:4, :], op=ALU.add)
                    t3 = gsb.tile([128, 1, 128], BF16, tag="th30")
                    nc.vector.tensor_tensor(out=t3, in0=t2[:, 0:1, :], in1=t2[:, 1:2, :], op=ALU.add)
                    nc.vector.tensor_tensor(out=xsum_h[:, ht, :], in0=xsum_h[:, ht, :],
                                            in1=t3[:, 0, :], op=ALU.add)
                pks = []
                for kt in (0, 1):
                    pk = ps.tile([128, 8, 128], F32, tag=f"{'AB'[wb % 2]}{kt}")
                    for bh in (0, 1):  # one PSUM bank (2KB) per matmul group
                        for ht in (0, 1):
                            nc.tensor.matmul(pk[:, bh * 4:bh * 4 + 4, :],
                                             lhsT=tfwd_t[:, ht, kt * 128:(kt + 1) * 128],
                                             rhs=xt[ht][:, bh * 4:bh * 4 + 4, :],
                                             start=(ht == 0), stop=(ht == 1))
                    pks.append(pk)
                for kt in (0, 1):
                    nc.vector.tensor_tensor(
                        out=ubuf_h[:, kt, w0:w0 + 8, :], in0=pks[kt],
                        in1=ghh[:, kt, :].unsqueeze(1).broadcast_to([128, 8, 128]),
                        op=ALU.mult)

            # ---------------- WC branch: spectrum + inverse -> ybuf_w ----------------
            for hb in range(32):
                h0 = hb * 8
                if hb == 1:
                    # HC pool partial = sum_{h, w-half} x -> single tiny
                    # AllReduce, issued here so PE flows into WC block 0 first
                    ph_ps = ps.tile([128, 1], F32, tag="B0")
                    for ht in (0, 1):
                        nc.tensor.matmul(ph_ps, lhsT=xsum_h[:, ht, :], rhs=ones_t,
                                         start=(ht == 0), stop=(ht == 1))
                    poolh_sb = gsb.tile([128, 1], F32, tag="poolh")
                    nc.vector.tensor_copy(poolh_sb, ph_ps)
                    nc.scalar.dma_start(out=arh_in[:], in_=poolh_sb)
                    nc.gpsimd.collective_compute(
                        "AllReduce", ALU.add,
                        replica_groups=[[0, 1], [2, 3], [4, 5], [6, 7]],
                        ins=[arh_in[:]], outs=[arh_out[:]])
                xt = []
                for wt in (0, 1):
                    t = xin.tile([128, 8, 128], BF16, tag=f"xb{wt}")
                    nc.sync.dma_start(out=t, in_=xw_d[wt * 128:(wt + 1) * 128, h0:h0 + 8, :])
                    xt.append(t)
                    t1 = gsb.tile([128, 4, 128], BF16, tag="th10")
                    nc.vector.tensor_tensor(out=t1, in0=t[:, 0:4, :], in1=t[:, 4:8, :], op=ALU.add)
                    t2 = gsb.tile([128, 2, 128], BF16, tag="th20")
                    nc.vector.tensor_tensor(out=t2, in0=t1[:, 0:2, :], in1=t1[:, 2:4, :], op=ALU.add)
                    t3 = gsb.tile([128, 1, 128], BF16, tag="th30")
                    nc.vector.tensor_tensor(out=t3, in0=t2[:, 0:1, :], in1=t2[:, 1:2, :], op=ALU.add)
                    nc.vector.tensor_tensor(out=xsum_w[:, wt, :], in0=xsum_w[:, wt, :],
                                            in1=t3[:, 0, :], op=ALU.add)
                uts = []
                for kt in (0, 1):
                    pk = ps.tile([128, 8, 128], F32, tag=f"A{kt}")
                    for bh in (0, 1):
                        for wt in (0, 1):
                            nc.tensor.matmul(pk[:, bh * 4:bh * 4 + 4, :],
                                             lhsT=tfwd_t[:, wt, kt * 128:(kt + 1) * 128],
                                             rhs=xt[wt][:, bh * 4:bh * 4 + 4, :],
                                             start=(wt == 0), stop=(wt == 1))
                    ut = uch.tile([128, 8, 128], BF16, tag=f"u{kt}")
                    nc.vector.tensor_tensor(
                        out=ut, in0=pk,
                        in1=ghw[:, kt, :].unsqueeze(1).broadcast_to([128, 8, 128]),
                        op=ALU.mult)
                    uts.append(ut)
                for half in (0, 1):
                    pw = ps.tile([128, 4, 128], F32, tag=f"B{half}")
                    for i in range(4):
                        hh = half * 4 + i
                        for kt in (0, 1):
                            nc.tensor.matmul(pw[:, i, :], lhsT=uts[kt][:, hh, :],
                                             rhs=tinvw_t[:, kt, :],
                                             start=(kt == 0), stop=(kt == 1))
                    nc.scalar.activation(ybuf_w[:, h0 + half * 4:h0 + half * 4 + 4, :],
                                         pw, AF.Copy)

            # ---------------- pools + channel attention -> folded conv weights ----
            pw_ps = ps.tile([128, 1], F32, tag="A0")
            for wt in (0, 1):
                nc.tensor.matmul(pw_ps, lhsT=xsum_w[:, wt, :], rhs=ones_t,
                                 start=(wt == 0), stop=(wt == 1))
            g0h_ps = ps.tile([128, 1], F32, tag="A1")
            nc.tensor.matmul(g0h_ps, lhsT=ghh[0:1, 0, :], rhs=one1_t, start=True, stop=True)
            g0w_ps = ps.tile([128, 1], F32, tag="B0")
            nc.tensor.matmul(g0w_ps, lhsT=ghw[0:1, 0, :], rhs=one1_t, start=True, stop=True)

            arh_sb = gsb.tile([128, 1], F32, tag="arhsb")
            nc.sync.dma_start(out=arh_sb, in_=arh_out[:])
            p_sb = []
            p0 = gsb.tile([128, 1], F32, tag="p0")
            nc.vector.tensor_tensor(out=p0, in0=arh_sb, in1=g0h_ps, op=ALU.mult)
            p_sb.append(p0)
            g0w_sb = gsb.tile([128, 1], F32, tag="g0wsb")
            nc.vector.tensor_copy(g0w_sb, g0w_ps)
            p1 = gsb.tile([128, 1], F32, tag="p1")
            nc.vector.tensor_tensor(out=p1, in0=pw_ps, in1=g0w_sb, op=ALU.mult)
            p_sb.append(p1)

            q_sb = []
            for ot in (0, 1):
                q_ps = ps.tile([128, 1], F32, tag=f"B{ot}")
                for ct in (0, 1):
                    nc.tensor.matmul(q_ps, lhsT=caw1t_t[:, ct, ot * 128:(ot + 1) * 128],
                                     rhs=p_sb[ct], start=(ct == 0), stop=(ct == 1))
                qt = gsb.tile([128, 1], F32, tag=f"q{ot}")
                nc.scalar.activation(qt, q_ps, AF.Gelu, bias=vec_t["cab1"][:, ot:ot + 1])
                nc.vector.tensor_tensor(out=qt, in0=qt, in1=vec_t["dwc"][:, ot:ot + 1],
                                        op=ALU.mult)
                q_sb.append(qt)
            s_sb = []
            for ot in (0, 1):
                s_t = gsb.tile([128, 1], F32, tag=f"s{ot}")
                nc.scalar.activation(s_t, q_sb[ot], AF.Sigmoid, bias=vec_t["dwb"][:, ot:ot + 1])
                s_sb.append(s_t)
            wsc = consts.tile([128, 2, 256], BF16, tag="wsc")
            for ct in (0, 1):
                nc.vector.tensor_scalar_mul(wsc[:, ct, :], lcwt_t[:, ct, :], s_sb[ct])

            # ---------------- conv 1x1 + BN + GELU + residual, streamed out ------
            for ch in range(32):
                w0 = ch * 4
                # JIT inverse of the HC spectrum: y_h columns, channel-major
                py = ps.tile([128, 4, 256], F32, tag=f"A{ch % 2}")
                for i in range(4):
                    for kt in (0, 1):
                        nc.tensor.matmul(py[:, i, :], lhsT=ubuf_h[:, kt, w0 + i, :],
                                         rhs=tinv_t[:, kt, :],
                                         start=(kt == 0), stop=(kt == 1))
                yt = stg.tile([128, 4, 256], BF16, tag="yt")
                nc.vector.tensor_copy(yt, py)
                xrs = []
                for ot in (0, 1):
                    xr = crhs.tile([128, 4, 256], BF16, tag=f"xr{ot}")
                    nc.sync.dma_start(out=xr,
                                      in_=xres_d[ot * 128:(ot + 1) * 128, w0:w0 + 4, :])
                    xrs.append(xr)
                for ot in (0, 1):
                    po = ps.tile([128, 4, 256], F32, tag=f"B{ot}")
                    for wh in (0, 1):
                        pos = po[:, wh * 2:wh * 2 + 2, :]
                        nc.tensor.matmul(pos, lhsT=wsc[:, 0, ot * 128:(ot + 1) * 128],
                                         rhs=yt[:, wh * 2:wh * 2 + 2, :],
                                         start=True, stop=False)
                        nc.tensor.matmul(pos, lhsT=wsc[:, 1, ot * 128:(ot + 1) * 128],
                                         rhs=ybuf_w[:, :, w0 + wh * 2:w0 + wh * 2 + 2]
                                         .transpose([0, 2, 1]),
                                         start=False, stop=True)
                    ga = outp.tile([128, 4, 256], BF16, tag=f"ga{ot}")
                    nc.scalar.activation(ga, po, AF.Gelu,
                                         bias=bnbeff[:, ot:ot + 1],
                                         scale=bninv[:, ot:ot + 1])
                    og = outp.tile([128, 4, 256], BF16, tag=f"og{ot}")
                    nc.vector.tensor_tensor(out=og, in0=ga, in1=xrs[ot], op=ALU.add)
                    eng = nc.scalar if ot == 0 else nc.sync
                    eng.dma_start(out=out_d[ot * 128:(ot + 1) * 128, w0:w0 + 4, :],
                                  in_=og)

    nc.compile()
    return nc


_NC_CACHE = None


def _get_nc():
    global _NC_CACHE
    if _NC_CACHE is None:
        _NC_CACHE = _build()
    return _NC_CACHE


def _host_consts(inputs, core):
    """Per-core constant inputs (everything except the x shards)."""
    s = core % 2
    wlo = WS * s
    T = _dft_basis()
    d = {}
    d["tfwd"] = _part_major(np.ascontiguousarray(T.T)).astype(_BF16_NP)
    d["tinv"] = _part_major(T).astype(_BF16_NP)
    d["tinvw"] = _part_major(np.ascontiguousarray(T[:, wlo:wlo + WS])).astype(_BF16_NP)
    d["omega"] = (np.arange(129, dtype=np.float32) / 128.0 - 1.0).reshape(1, 129)
    d["lam"] = np.linspace(-1.0, 1.0, 128, dtype=np.float32).reshape(1, 128)
    for m, (ha, hb) in (("o", ("ah", "aw")), ("l", ("bc1", "bc2"))):
        w1 = np.zeros((1, 128), np.float32)
        w1[0, 0:64] = inputs[f"{ha}_w1"][:, 0]
        w1[0, 64:128] = inputs[f"{hb}_w1"][:, 0]
        d[f"{m}_w1t"] = w1
        b1 = np.zeros((128, 1), np.float32)
        b1[0:64, 0] = inputs[f"{ha}_b1"]
        b1[64:128, 0] = inputs[f"{hb}_b1"]
        d[f"{m}_b1v"] = b1
        w2 = np.zeros((128, 128), np.float32)
        w2[0:64, 0:64] = inputs[f"{ha}_w2"].T
        w2[64:128, 64:128] = inputs[f"{hb}_w2"].T
        d[f"{m}_w2t"] = w2
        b2 = np.zeros((128, 1), np.float32)
        b2[0:64, 0] = inputs[f"{ha}_b2"]
        b2[64:128, 0] = inputs[f"{hb}_b2"]
        d[f"{m}_b2v"] = b2
        w3 = np.zeros((128, 40), np.float32)
        w3[0:64, 0:8] = inputs[f"{ha}_w3"].T
        w3[64:128, 32:40] = inputs[f"{hb}_w3"].T
        d[f"{m}_w3t"] = w3
        b3 = np.zeros((40, 1), np.float32)
        b3[0:8, 0] = inputs[f"{ha}_b3"]
        b3[32:40, 0] = inputs[f"{hb}_b3"]
        d[f"{m}_b3v"] = b3
    d["caw1t"] = _part_major(np.ascontiguousarray(inputs["ca_w1"].T) / 65536.0).astype(np.float32)
    d["cab1"] = _part_major(inputs["ca_b1"]).astype(np.float32)
    d["dwc"] = _part_major(np.ascontiguousarray(inputs["ca_dw"][:, 1, 1])).astype(np.float32)
    d["dwb"] = _part_major(inputs["ca_db"]).astype(np.float32)
    d["lcwt"] = _part_major(np.ascontiguousarray(inputs["lc_w"].T)).astype(np.float32)
    d["bng"] = _part_major(inputs["bn_g"]).astype(np.float32)
    d["bnb"] = _part_major(inputs["bn_b"]).astype(np.float32)
    d["bnm"] = _part_major(inputs["bn_m"]).astype(np.float32)
    d["bnv"] = _part_major(inputs["bn_v"]).astype(np.float32)
    return d


def kernel(**inputs):
    x = np.asarray(inputs["x"], np.float32)
    nc = _get_nc()

    in_maps = []
    for core in range(NCORES):
        b, s = core // 2, core % 2
        wlo = WS * s
        m = _host_consts(inputs, core)
        # xh[h, w, c] = x[b, c, h, wlo+w]
        m["xh"] = np.ascontiguousarray(
            x[b, :C2, :, wlo:wlo + WS].transpose(1, 2, 0)).astype(_BF16_NP)
        # xw[w, h, c] = x[b, 128+c, h, w]
        m["xw"] = np.ascontiguousarray(
            x[b, C2:, :, :].transpose(2, 1, 0)).astype(_BF16_NP)
        # xres[c, w, h] = x[b, c, h, wlo+w]
        m["xres"] = np.ascontiguousarray(
            x[b, :, :, wlo:wlo + WS].transpose(0, 2, 1)).astype(_BF16_NP)
        in_maps.append(m)

    trace = os.environ.get("BASS_KERNEL_TRACE", "0") == "1"
    res = bass_utils.run_bass_kernel_spmd(
        nc, in_maps, core_ids=list(range(NCORES)),
        trace=trace, trace_cores=list(range(NCORES)) if trace else None,
        stitch_traces=False)
    if trace and res.exec_time_ns is not None:
        print(f"HW exec time: {res.exec_time_ns} ns")
        print(f"   mean exec time: {res.mean_exec_time_ns} ns  "
              f"(slowest core {res.max_exec_time_core_id})")
        if res.instructions_and_trace is not None:
            print("   trace:", res.instructions_and_trace[1])

    out = np.empty((B, 2 * C2, N, N), np.float32)
    for core in range(NCORES):
        b, s = core // 2, core % 2
        wlo = WS * s
        # device out is [c, w, h] bf16
        out[b, :, :, wlo:wlo + WS] = np.asarray(
            res.results[core]["out"], np.float32).transpose(0, 2, 1)
    return out
